# revision 16
# baseline (speedup 1.0000x reference)
"""Self-contained TRN2 Bass kernel for the causal multi-head attention problem.

Problem (hardcoded): B=2, S=2048, D=1024, H=16, DH=64, fp32, causal.
Sharding: 8 cores = 2 batches x 4 head-groups of 4 heads each.

Per-core strategy ("T layout": feature dim on partitions, sequence on free):
  x ships as an fp8 hi+lo pair (host split); Q/K/V projections run as
  error-compensated fp8 DoubleRow matmuls (W_hi*x_hi + W_hi*x_lo + W_lo*x_hi
  into one fp32 PSUM group, K=256 per matmul) -- 2x fewer PE cycles than
  fp32r at ~bf16 accuracy. Attention runs bf16 (qT/kT/et/vext) at exact
  causal widths; output projection stays fp32r. Scores for one sk-chunk put
  both heads of a pair side by side in a [128,1024] PSUM tile so one
  activation does the exp for both; the sk>sq triangles of diagonal chunks
  are zeroed by 0/1 bf16 multiplies (head 0 on DVE for latency, head 1 on
  the otherwise-idle Pool engine). Softmax denominator rides the z matmul
  as a ones-column; division via K=33 ones-matmul broadcast + DVE
  reciprocal + multiply.

Scheduling: scores run one sk-chunk ahead of z (skew) so the ACT-engine exp
latency is hidden, and a deadline-driven filler queue interleaves projection
and output-projection PSUM groups between attention chunks (with priority
at diagonal chunks, whose exp->mask->z chains are longest) so the PE never
idles. Host folds: 1/sqrt(DH) and the fp8 weight scale into the PSUM
evacuation scale, b_Q/b_K into the evacuation bias, b_V and b_O into a
single host-side output bias (rows of attn sum to 1); final partial sums
over the 4 head-group cores on the host.
"""

import numpy as np

B, S, D = 2, 2048, 1024
H, DH = 16, 64
ATTN_SCALE = 8.0  # sqrt(64)
WS = 64.0         # fp8 weight pre-scale (folded out at PSUM evacuation)
N_CORES = 8
NC = D // 128          # 8 D-chunks
NB = S // 512          # 4 sq bands
NSK = S // 128         # 16 sk chunks

_COMPILED = None


def _build_program():
    import concourse.mybir as mybir
    import concourse.tile as tile
    from concourse import bacc

    F32 = mybir.dt.float32
    F32R = mybir.dt.float32r
    BF16 = mybir.dt.bfloat16
    FP8 = mybir.dt.float8e4
    DR = mybir.MatmulPerfMode.DoubleRow
    AF = mybir.ActivationFunctionType
    ALU = mybir.AluOpType

    nc = bacc.Bacc("TRN2", target_bir_lowering=False, debug=False,
                   num_devices=N_CORES)

    xh = nc.dram_tensor("xh", [128, NC, S], FP8, kind="ExternalInput")
    xl = nc.dram_tensor("xl", [128, NC, S], FP8, kind="ExternalInput")
    wqh = nc.dram_tensor("wqh", [128, 2, 4, 2, 128], FP8, kind="ExternalInput")
    wql = nc.dram_tensor("wql", [128, 2, 4, 2, 128], FP8, kind="ExternalInput")
    wkh = nc.dram_tensor("wkh", [128, 2, 4, 2, 128], FP8, kind="ExternalInput")
    wkl = nc.dram_tensor("wkl", [128, 2, 4, 2, 128], FP8, kind="ExternalInput")
    wvh = nc.dram_tensor("wvh", [128, 4, 2, 256], FP8, kind="ExternalInput")
    wvl = nc.dram_tensor("wvl", [128, 4, 2, 256], FP8, kind="ExternalInput")
    wo = nc.dram_tensor("wo", [128, 2, NC, 128], F32R, kind="ExternalInput")
    bq = nc.dram_tensor("bq", [128, 2], F32, kind="ExternalInput")
    bk = nc.dram_tensor("bk", [128, 2], F32, kind="ExternalInput")
    ones2 = nc.dram_tensor("ones2", [33, 128], F32R, kind="ExternalInput")
    onesv = nc.dram_tensor("onesv", [128, NSK, 4, 1], BF16, kind="ExternalInput")
    mtri = nc.dram_tensor("mtri", [128, 128], BF16, kind="ExternalInput")
    ot = nc.dram_tensor("ot", [NC, 128, S], F32, kind="ExternalOutput")

    with tile.TileContext(nc) as tc:
        with (
            tc.tile_pool(name="const", bufs=1) as cst,
            tc.tile_pool(name="xbp", bufs=8) as xbp,
            tc.tile_pool(name="qkz", bufs=1) as qkz,
            tc.tile_pool(name="expp", bufs=6) as expp,
            tc.tile_pool(name="rowp", bufs=3) as rowp,
            tc.tile_pool(name="rbp", bufs=3) as rbp,
            tc.tile_pool(name="outp", bufs=4) as outp,
            tc.tile_pool(name="pss", bufs=2, space="PSUM") as pss,
            tc.tile_pool(name="psw", bufs=2, space="PSUM") as psw,
            tc.tile_pool(name="psz", bufs=2, space="PSUM") as psz,
        ):
            wqh_sb = cst.tile([128, 2, 4, 2, 128], FP8)
            wql_sb = cst.tile([128, 2, 4, 2, 128], FP8)
            wkh_sb = cst.tile([128, 2, 4, 2, 128], FP8)
            wkl_sb = cst.tile([128, 2, 4, 2, 128], FP8)
            wvh_sb = cst.tile([128, 4, 2, 256], FP8)
            wvl_sb = cst.tile([128, 4, 2, 256], FP8)
            wo_sb = cst.tile([128, 2, NC, 128], F32R)
            bq_sb = cst.tile([128, 2], F32)
            bk_sb = cst.tile([128, 2], F32)
            on2_sb = cst.tile([33, 128], F32R)
            mtri_sb = cst.tile([128, 128], BF16)
            xhb = [xbp.tile([128, NC, 512], FP8, name=f"xhb{j}", tag="xb")
                   for j in range(NB)]
            xlb = [xbp.tile([128, NC, 512], FP8, name=f"xlb{j}", tag="xb")
                   for j in range(NB)]
            qT = qkz.tile([128, 2, S], BF16)    # [2 heads of pair x dh, pr, s]
            kT = qkz.tile([128, 2, S], BF16)
            vext = qkz.tile([128, NSK, 4, 65], BF16)  # [sk, chunk, head, dh|1]
            zT = qkz.tile([128, 2, S], F32R)

            # warm the PE (p-state) and the ACT exp table while the input
            # DMAs are in flight; results are discarded
            wu_w = cst.tile([128, 128], F32)
            wu_r = cst.tile([128, 512], F32)
            wu_o = cst.tile([128, 512], F32)
            nc.vector.memset(wu_w[:], 0.0)
            nc.vector.memset(wu_r[:], 0.0)
            wup = psw.tile([128, 512], F32, tag="w", name="wup")
            for _i in range(8):
                nc.tensor.matmul(wup[:], wu_w[:], wu_r[:],
                                 start=(_i == 0), stop=(_i == 7))
            nc.scalar.activation(wu_o[:], wu_r[:], AF.Exp)

            # DMA order: first-proj-unit critical path first
            nc.sync.dma_start(out=wqh_sb[:], in_=wqh[:])
            nc.sync.dma_start(out=xhb[0][:], in_=xh[:, :, 0:512])
            nc.sync.dma_start(out=wql_sb[:], in_=wql[:])
            nc.sync.dma_start(out=xlb[0][:], in_=xl[:, :, 0:512])
            nc.sync.dma_start(out=wkh_sb[:], in_=wkh[:])
            nc.sync.dma_start(out=wkl_sb[:], in_=wkl[:])
            nc.sync.dma_start(out=bq_sb[:], in_=bq[:])
            nc.sync.dma_start(out=bk_sb[:], in_=bk[:])
            nc.sync.dma_start(out=wvh_sb[:], in_=wvh[:])
            nc.sync.dma_start(out=wvl_sb[:], in_=wvl[:])
            nc.sync.dma_start(out=mtri_sb[:], in_=mtri[:])
            nc.sync.dma_start(out=on2_sb[:], in_=ones2[:])
            nc.sync.dma_start(out=vext[:, :, :, 64:65], in_=onesv[:])
            for j in range(1, NB):
                nc.sync.dma_start(out=xhb[j][:], in_=xh[:, :, j * 512:(j + 1) * 512])
                nc.sync.dma_start(out=xlb[j][:], in_=xl[:, :, j * 512:(j + 1) * 512])
            nc.sync.dma_start(out=wo_sb[:], in_=wo[:])

            # ---------- emit-unit definitions ----------

            def emit_qk(jj, pr, is_q):
                js = slice(jj * 512, (jj + 1) * 512)
                wh, wl = (wqh_sb, wql_sb) if is_q else (wkh_sb, wkl_sb)
                ps = psw.tile([128, 512], F32, tag="w",
                              name=f"qk{jj}{pr}{int(is_q)}")
                i = 0
                for w_sb, x_b in ((wh, xhb[jj]), (wh, xlb[jj]), (wl, xhb[jj])):
                    for kp in range(4):
                        nc.tensor.matmul(
                            ps[:], w_sb[:, pr, kp],
                            x_b[:, 2 * kp:2 * kp + 2, :],
                            start=(i == 0), stop=(i == 11), perf_mode=DR,
                        )
                        i += 1
                if is_q:
                    nc.vector.tensor_scalar(
                        qT[:, pr, js], ps[:], 1.0 / (WS * ATTN_SCALE),
                        bq_sb[:, pr:pr + 1], ALU.mult, ALU.add)
                else:
                    nc.vector.tensor_scalar(
                        kT[:, pr, js], ps[:], 1.0 / WS,
                        bk_sb[:, pr:pr + 1], ALU.mult, ALU.add)

            def emit_v(jj, sl):
                sk = 4 * jj + sl
                ss = slice(sl * 128, (sl + 1) * 128)
                ps = psw.tile([128, 256], F32, tag="w", name=f"v{jj}{sl}")
                i = 0
                for x_b, w_sb in ((xhb[jj], wvh_sb), (xlb[jj], wvh_sb),
                                  (xhb[jj], wvl_sb)):
                    for kp in range(4):
                        nc.tensor.matmul(
                            ps[:], x_b[:, 2 * kp:2 * kp + 2, ss],
                            w_sb[:, kp], start=(i == 0), stop=(i == 11),
                            perf_mode=DR,
                        )
                        i += 1
                nc.vector.tensor_scalar(
                    vext[:, sk, :, 0:64],
                    ps[:].rearrange("p (h d) -> p h d", h=4),
                    1.0 / WS, None, ALU.mult)

            et_live = {}

            def emit_scores(j, pr, c):
                # one sk-chunk, both heads side by side: hh0 -> [lo:512],
                # hh1 -> [512+lo:1024]; one exp covers both via a strided
                # view; diagonal triangles zeroed by bf16 mtri multiplies
                r = c - 4 * j
                lo = 0 if r < 0 else r * 128
                sp = pss.tile([128, 1024], F32, tag="s", name=f"s{j}{pr}{c}")
                for hh in range(2):
                    hp = slice(64 * hh, 64 * hh + 64)
                    nc.tensor.matmul(
                        sp[:, 512 * hh + lo:512 * hh + 512],
                        kT[hp, pr, c * 128:(c + 1) * 128],
                        qT[hp, pr, j * 512 + lo:(j + 1) * 512],
                        start=True, stop=True)
                et = expp.tile([128, 1024], BF16, tag="et", name=f"e{j}{pr}{c}")
                if lo:
                    ev = et.rearrange("p (t f) -> p t f", t=2)
                    sv = sp.rearrange("p (t f) -> p t f", t=2)
                    nc.scalar.activation(ev[:, :, lo:512], sv[:, :, lo:512],
                                         AF.Exp)
                else:
                    nc.scalar.activation(et[:], sp[:], AF.Exp)
                if r >= 0:
                    # masks on the Pool engine: keeps them out of the DVE
                    # queue, which carries the latency-critical division ops
                    nc.gpsimd.tensor_mul(
                        et[:, lo:lo + 128], et[:, lo:lo + 128], mtri_sb[:])
                    nc.gpsimd.tensor_mul(
                        et[:, 512 + lo:512 + lo + 128],
                        et[:, 512 + lo:512 + lo + 128], mtri_sb[:])
                et_live[(pr, c)] = (et, lo)

            zps_live = {}

            def emit_z(j, pr, c):
                nsk = 4 * (j + 1)
                et, lo = et_live.pop((pr, c))
                if c == 0:
                    zps_live[pr] = [
                        psz.tile([65, 512], F32, tag="z", name=f"z{j}{pr}{hh}")
                        for hh in range(2)]
                zps = zps_live[pr]
                for hh in range(2):
                    nc.tensor.matmul(
                        zps[hh][:, lo:512], vext[:, c, 2 * pr + hh, :],
                        et[:, 512 * hh + lo:512 * hh + 512],
                        start=(c == 0), stop=(c == nsk - 1))

            def emit_division(j, pr):
                js = slice(j * 512, (j + 1) * 512)
                zps = zps_live.pop(pr)
                # denominator rows at partitions 0/32; K=33 ones-matmul
                # broadcasts h0 -> rows 0-63, h1 -> rows 64-127
                rows = rowp.tile([33, 512], F32R, tag="row", name=f"r{j}{pr}")
                nc.vector.tensor_copy(rows[0:1, :], zps[0][64:65, :])
                nc.vector.tensor_copy(rows[32:33, :], zps[1][64:65, :])
                bcp = psw.tile([128, 512], F32, tag="w", name=f"b{j}{pr}")
                nc.tensor.matmul(bcp[:], on2_sb[:], rows[:],
                                 start=True, stop=True)
                rb = rbp.tile([128, 512], F32, tag="rb", name=f"rb{j}{pr}")
                nc.vector.reciprocal(rb[:], bcp[:])
                for hh in range(2):
                    hp = slice(64 * hh, 64 * hh + 64)
                    nc.vector.tensor_mul(zT[hp, pr, js], zps[hh][0:64, :],
                                         rb[hp, :])

            def emit_out(j, c):
                js = slice(j * 512, (j + 1) * 512)
                ops = psw.tile([128, 512], F32, tag="w", name=f"o{j}{c}")
                for pr in range(2):
                    nc.tensor.matmul(
                        ops[:], wo_sb[:, pr, c, :], zT[:, pr, js],
                        start=(pr == 0), stop=(pr == 1))
                ob = outp.tile([128, 512], F32, tag="ob", name=f"ob{j}{c}")
                # evacuate on ACT (has headroom; DVE queue stays short)
                nc.scalar.activation(ob[:], ops[:], AF.Copy)
                nc.sync.dma_start(out=ot[c, :, js], in_=ob[:])

            # ---------- deadline-driven scheduler ----------
            import heapq
            Q = []
            seq = [0]
            BIG = 1 << 30

            def key(j, pr, c):
                return (j * 2 + pr) * 100 + c

            def push(dl, fn):
                seq[0] += 1
                heapq.heappush(Q, (dl, seq[0], fn))

            # deadlines sit one context EARLY (-150/-120/-60) so boundary
            # work (next band's q/k) is emitted before the division chain
            # serializes the pr transition
            for jj in range(NB):
                for pr in range(2):
                    push(key(jj, pr, 0) - 150,
                         (lambda a=jj, b=pr: emit_qk(a, b, True)))
                    push(key(jj, pr, 4 * jj) - 120,
                         (lambda a=jj, b=pr: emit_qk(a, b, False)))
                for sl in range(4):
                    push(key(jj, 0, 4 * jj + sl) - 60,
                         (lambda a=jj, b=sl: emit_v(a, b)))

            credit = [0.0]
            rem_pumps = [sum(2 * (2 * 4 * (j + 1) + 1) for j in range(NB))]

            def pump(pos, force=0):
                while Q and Q[0][0] <= pos:
                    heapq.heappop(Q)[2]()
                rate = min(2.0, len(Q) / max(1, rem_pumps[0]))
                rem_pumps[0] -= 1
                credit[0] += rate
                n = 0
                while Q and (credit[0] >= 1.0 or n < force):
                    heapq.heappop(Q)[2]()
                    if credit[0] >= 1.0:
                        credit[0] -= 1.0
                    n += 1

            def flush_div(pending):
                dj, dpr = pending
                emit_division(dj, dpr)
                if dpr == 1:
                    # BIG deadline: the heap then prefers every projection
                    # unit over out units, so outs drift to the filler-
                    # starved late bands on their own
                    for c in range(NC):
                        push(BIG, (lambda a=dj, b=c: emit_out(a, b)))

            pending_div = None
            for j in range(NB):
                G = 4 * (j + 1)
                for pr in range(2):
                    pump(key(j, pr, 0))
                    emit_scores(j, pr, 0)
                    if pending_div is not None:
                        flush_div(pending_div)
                        pending_div = None
                    for g in range(G):
                        if g + 1 < G:
                            nxt = g + 1
                            diag = (nxt - 4 * j) >= 0
                            pump(key(j, pr, nxt), force=1 if diag else 0)
                            emit_scores(j, pr, nxt)
                        else:
                            pump(key(j, pr, g))
                        emit_z(j, pr, g)
                        pump(key(j, pr, g))
                    pending_div = (j, pr)
            flush_div(pending_div)
            while Q:
                heapq.heappop(Q)[2]()

    nc.compile()
    return nc


def _mtri():
    p = np.arange(128)[:, None]
    f = np.arange(128)[None, :]
    return (f >= p)


def _ones2():
    o = np.zeros((33, 128), np.float32)
    o[0, 0:64] = 1.0
    o[32, 64:128] = 1.0
    return o


_XT_CACHE = {}
_W_CACHE = {}


def _split8(a):
    import ml_dtypes
    E4 = ml_dtypes.float8_e4m3
    hi = np.asarray(a, np.float32).astype(E4)
    lo = (np.asarray(a, np.float32) - hi.astype(np.float32)).astype(E4)
    return hi, lo


def _prep_core(core, x, W_Q, W_K, W_V, W_O, b_Q, b_K):
    import ml_dtypes
    BF = ml_dtypes.bfloat16
    b, g = divmod(core, 4)
    h0 = 4 * g
    key = id(x)
    if (key, b) not in _XT_CACHE:
        if len(_XT_CACHE) > 8:
            _XT_CACHE.clear()
        xT = np.ascontiguousarray(x[b].T)                 # [D, S]
        xp = np.ascontiguousarray(xT.reshape(NC, 128, S).transpose(1, 0, 2))
        _XT_CACHE[(key, b)] = _split8(xp)
    xh, xl = _XT_CACHE[(key, b)]

    wkey = (id(W_Q), g)
    if wkey not in _W_CACHE:
        if len(_W_CACHE) > 8:
            _W_CACHE.clear()

        def pack_qk(W):
            out = np.empty((128, 2, 4, 2, 128), np.float32)
            for pr in range(2):
                Wp = WS * W[h0 + 2 * pr:h0 + 2 * pr + 2].reshape(128, D)
                # [m, D] -> [p, kp, kt, m]
                out[:, pr] = Wp.T.reshape(4, 2, 128, 128).transpose(2, 0, 1, 3)
            return _split8(out)

        Wv4 = WS * W_V[h0:h0 + 4].reshape(256, D)         # [n, D]
        wv = np.ascontiguousarray(
            Wv4.T.reshape(4, 2, 128, 256).transpose(2, 0, 1, 3))

        wo = np.empty((128, 2, NC, 128), np.float32)
        for pr in range(2):
            Wp = W_O[h0 + 2 * pr:h0 + 2 * pr + 2]         # [2, D, 64]
            arr = Wp.transpose(0, 2, 1).reshape(128, D)   # [128(k), D]
            wo[:, pr] = arr.reshape(128, NC, 128)
        wo = np.ascontiguousarray(wo)

        _W_CACHE[wkey] = (pack_qk(W_Q), pack_qk(W_K), _split8(wv), wo)
    (wqh, wql), (wkh, wkl), (wvh, wvl), wo = _W_CACHE[wkey]

    bqv = np.stack([b_Q[h0 + 2 * pr:h0 + 2 * pr + 2].reshape(128) / ATTN_SCALE
                    for pr in range(2)], axis=1).astype(np.float32)
    bkv = np.stack([b_K[h0 + 2 * pr:h0 + 2 * pr + 2].reshape(128)
                    for pr in range(2)], axis=1).astype(np.float32)

    return dict(
        xh=xh, xl=xl, wqh=wqh, wql=wql, wkh=wkh, wkl=wkl,
        wvh=wvh, wvl=wvl, wo=wo,
        bq=bqv, bk=bkv,
        ones2=_ones2(),
        mtri=_mtri().astype(BF),
        onesv=np.ones((128, NSK, 4, 1), np.float32).astype(BF),
    )


def kernel(x, W_Q, W_K, W_V, W_O, b_Q, b_K, b_V, b_O):
    global _COMPILED
    from concourse.bass_utils import run_bass_kernel_spmd

    x = np.asarray(x, np.float32)
    W_Q = np.asarray(W_Q, np.float32)
    W_K = np.asarray(W_K, np.float32)
    W_V = np.asarray(W_V, np.float32)
    W_O = np.asarray(W_O, np.float32)
    b_Q = np.asarray(b_Q, np.float32)
    b_K = np.asarray(b_K, np.float32)
    b_V = np.asarray(b_V, np.float32)
    b_O = np.asarray(b_O, np.float32)

    if _COMPILED is None:
        _COMPILED = _build_program()
    nc = _COMPILED

    in_maps = [_prep_core(c, x, W_Q, W_K, W_V, W_O, b_Q, b_K)
               for c in range(N_CORES)]
    res = run_bass_kernel_spmd(nc, in_maps, core_ids=list(range(N_CORES)))

    # host gather: sum head-group partials, add folded output bias, transpose
    bias_total = b_O + np.einsum('idh,ih->d', W_O, b_V)
    out = np.empty((B, S, D), np.float32)
    for b in range(B):
        acc = res.results[4 * b]["ot"].astype(np.float64)
        for g in range(1, 4):
            acc += res.results[4 * b + g]["ot"]
        out[b] = acc.reshape(D, S).T + bias_total
    return out


# revision 17
# speedup vs baseline: 1.0321x; 1.0321x over previous
"""Self-contained TRN2 Bass kernel for the causal multi-head attention problem.

Problem (hardcoded): B=2, S=2048, D=1024, H=16, DH=64, fp32, causal.
Sharding: 8 cores = 2 batches x 4 head-groups of 4 heads each.

Per-core strategy ("T layout": feature dim on partitions, sequence on free):
  x ships as an fp8 hi+lo pair (host split); Q/K/V projections run as
  error-compensated fp8 DoubleRow matmuls (W_hi*x_hi + W_hi*x_lo + W_lo*x_hi
  into one fp32 PSUM group, K=256 per matmul) -- 2x fewer PE cycles than
  fp32r at ~bf16 accuracy. Attention runs bf16 (qT/kT/et/vext) at exact
  causal widths; output projection stays fp32r. Scores for one sk-chunk put
  both heads of a pair side by side in a [128,1024] PSUM tile so one
  activation does the exp for both; the sk>sq triangles of diagonal chunks
  are zeroed by 0/1 bf16 multiplies (head 0 on DVE for latency, head 1 on
  the otherwise-idle Pool engine). Softmax denominator rides the z matmul
  as a ones-column; division via K=33 ones-matmul broadcast + DVE
  reciprocal + multiply.

Scheduling: scores run one sk-chunk ahead of z (skew) so the ACT-engine exp
latency is hidden, and a deadline-driven filler queue interleaves projection
and output-projection PSUM groups between attention chunks (with priority
at diagonal chunks, whose exp->mask->z chains are longest) so the PE never
idles. Host folds: 1/sqrt(DH) and the fp8 weight scale into the PSUM
evacuation scale, b_Q/b_K into the evacuation bias, b_V and b_O into a
single host-side output bias (rows of attn sum to 1); final partial sums
over the 4 head-group cores on the host.
"""

import numpy as np

B, S, D = 2, 2048, 1024
H, DH = 16, 64
ATTN_SCALE = 8.0  # sqrt(64)
WS = 64.0         # fp8 weight pre-scale (folded out at PSUM evacuation)
N_CORES = 8
NC = D // 128          # 8 D-chunks
NB = S // 512          # 4 sq bands
NSK = S // 128         # 16 sk chunks

_COMPILED = None


def _build_program():
    import concourse.mybir as mybir
    import concourse.tile as tile
    from concourse import bacc

    F32 = mybir.dt.float32
    F32R = mybir.dt.float32r
    BF16 = mybir.dt.bfloat16
    FP8 = mybir.dt.float8e4
    DR = mybir.MatmulPerfMode.DoubleRow
    AF = mybir.ActivationFunctionType
    ALU = mybir.AluOpType

    nc = bacc.Bacc("TRN2", target_bir_lowering=False, debug=False,
                   num_devices=N_CORES)

    xh = nc.dram_tensor("xh", [128, NC, S], FP8, kind="ExternalInput")
    xl = nc.dram_tensor("xl", [128, NC, S], FP8, kind="ExternalInput")
    wqh = nc.dram_tensor("wqh", [128, 2, 4, 2, 128], FP8, kind="ExternalInput")
    wql = nc.dram_tensor("wql", [128, 2, 4, 2, 128], FP8, kind="ExternalInput")
    wkh = nc.dram_tensor("wkh", [128, 2, 4, 2, 128], FP8, kind="ExternalInput")
    wkl = nc.dram_tensor("wkl", [128, 2, 4, 2, 128], FP8, kind="ExternalInput")
    wvh = nc.dram_tensor("wvh", [128, 4, 2, 256], FP8, kind="ExternalInput")
    wvl = nc.dram_tensor("wvl", [128, 4, 2, 256], FP8, kind="ExternalInput")
    wo = nc.dram_tensor("wo", [128, 2, NC, 128], F32R, kind="ExternalInput")
    bq = nc.dram_tensor("bq", [128, 2], F32, kind="ExternalInput")
    bk = nc.dram_tensor("bk", [128, 2], F32, kind="ExternalInput")
    ones2 = nc.dram_tensor("ones2", [33, 128], F32R, kind="ExternalInput")
    onesv = nc.dram_tensor("onesv", [128, NSK, 4, 1], BF16, kind="ExternalInput")
    mtri = nc.dram_tensor("mtri", [128, 128], BF16, kind="ExternalInput")
    ot = nc.dram_tensor("ot", [NC, 128, S], F32, kind="ExternalOutput")

    with tile.TileContext(nc) as tc:
        with (
            tc.tile_pool(name="const", bufs=1) as cst,
            tc.tile_pool(name="xbp", bufs=8) as xbp,
            tc.tile_pool(name="qkz", bufs=1) as qkz,
            tc.tile_pool(name="expp", bufs=6) as expp,
            tc.tile_pool(name="rowp", bufs=3) as rowp,
            tc.tile_pool(name="rbp", bufs=3) as rbp,
            tc.tile_pool(name="outp", bufs=4) as outp,
            tc.tile_pool(name="pss", bufs=2, space="PSUM") as pss,
            tc.tile_pool(name="psw", bufs=2, space="PSUM") as psw,
            tc.tile_pool(name="psz", bufs=2, space="PSUM") as psz,
        ):
            wqh_sb = cst.tile([128, 2, 4, 2, 128], FP8)
            wql_sb = cst.tile([128, 2, 4, 2, 128], FP8)
            wkh_sb = cst.tile([128, 2, 4, 2, 128], FP8)
            wkl_sb = cst.tile([128, 2, 4, 2, 128], FP8)
            wvh_sb = cst.tile([128, 4, 2, 256], FP8)
            wvl_sb = cst.tile([128, 4, 2, 256], FP8)
            wo_sb = cst.tile([128, 2, NC, 128], F32R)
            bq_sb = cst.tile([128, 2], F32)
            bk_sb = cst.tile([128, 2], F32)
            on2_sb = cst.tile([33, 128], F32R)
            mtri_sb = cst.tile([128, 128], BF16)
            xhb = [xbp.tile([128, NC, 512], FP8, name=f"xhb{j}", tag="xb")
                   for j in range(NB)]
            xlb = [xbp.tile([128, NC, 512], FP8, name=f"xlb{j}", tag="xb")
                   for j in range(NB)]
            qT = qkz.tile([128, 2, S], BF16)    # [2 heads of pair x dh, pr, s]
            kT = qkz.tile([128, 2, S], BF16)
            vext = qkz.tile([128, NSK, 4, 65], BF16)  # [sk, chunk, head, dh|1]
            zT = qkz.tile([128, 2, S], F32R)

            # warm the PE (p-state) and the ACT exp table while the input
            # DMAs are in flight; results are discarded
            wu_w = cst.tile([128, 128], F32)
            wu_r = cst.tile([128, 512], F32)
            wu_o = cst.tile([128, 512], F32)
            nc.vector.memset(wu_w[:], 0.0)
            nc.vector.memset(wu_r[:], 0.0)
            wup = psw.tile([128, 512], F32, tag="w", name="wup")
            for _i in range(8):
                nc.tensor.matmul(wup[:], wu_w[:], wu_r[:],
                                 start=(_i == 0), stop=(_i == 7))
            nc.scalar.activation(wu_o[:], wu_r[:], AF.Exp)

            # DMA order: first-proj-unit critical path first
            nc.sync.dma_start(out=wqh_sb[:], in_=wqh[:])
            nc.sync.dma_start(out=xhb[0][:], in_=xh[:, :, 0:512])
            nc.sync.dma_start(out=wql_sb[:], in_=wql[:])
            nc.sync.dma_start(out=xlb[0][:], in_=xl[:, :, 0:512])
            nc.sync.dma_start(out=wkh_sb[:], in_=wkh[:])
            nc.sync.dma_start(out=wkl_sb[:], in_=wkl[:])
            nc.sync.dma_start(out=bq_sb[:], in_=bq[:])
            nc.sync.dma_start(out=bk_sb[:], in_=bk[:])
            nc.sync.dma_start(out=wvh_sb[:], in_=wvh[:])
            nc.sync.dma_start(out=wvl_sb[:], in_=wvl[:])
            nc.sync.dma_start(out=mtri_sb[:], in_=mtri[:])
            nc.sync.dma_start(out=on2_sb[:], in_=ones2[:])
            nc.sync.dma_start(out=vext[:, :, :, 64:65], in_=onesv[:])
            for j in range(1, NB):
                nc.sync.dma_start(out=xhb[j][:], in_=xh[:, :, j * 512:(j + 1) * 512])
                nc.sync.dma_start(out=xlb[j][:], in_=xl[:, :, j * 512:(j + 1) * 512])
            nc.sync.dma_start(out=wo_sb[:], in_=wo[:])

            # ---------- emit-unit definitions ----------

            def emit_qk(jj, pr, is_q):
                js = slice(jj * 512, (jj + 1) * 512)
                wh, wl = (wqh_sb, wql_sb) if is_q else (wkh_sb, wkl_sb)
                ps = psw.tile([128, 512], F32, tag="w",
                              name=f"qk{jj}{pr}{int(is_q)}")
                i = 0
                for w_sb, x_b in ((wh, xhb[jj]), (wh, xlb[jj]), (wl, xhb[jj])):
                    for kp in range(4):
                        nc.tensor.matmul(
                            ps[:], w_sb[:, pr, kp],
                            x_b[:, 2 * kp:2 * kp + 2, :],
                            start=(i == 0), stop=(i == 11), perf_mode=DR,
                        )
                        i += 1
                if is_q:
                    nc.vector.tensor_scalar(
                        qT[:, pr, js], ps[:], 1.0 / (WS * ATTN_SCALE),
                        bq_sb[:, pr:pr + 1], ALU.mult, ALU.add)
                else:
                    nc.vector.tensor_scalar(
                        kT[:, pr, js], ps[:], 1.0 / WS,
                        bk_sb[:, pr:pr + 1], ALU.mult, ALU.add)

            def emit_v(jj, sl):
                sk = 4 * jj + sl
                ss = slice(sl * 128, (sl + 1) * 128)
                ps = psw.tile([128, 256], F32, tag="w", name=f"v{jj}{sl}")
                i = 0
                for x_b, w_sb in ((xhb[jj], wvh_sb), (xlb[jj], wvh_sb),
                                  (xhb[jj], wvl_sb)):
                    for kp in range(4):
                        nc.tensor.matmul(
                            ps[:], x_b[:, 2 * kp:2 * kp + 2, ss],
                            w_sb[:, kp], start=(i == 0), stop=(i == 11),
                            perf_mode=DR,
                        )
                        i += 1
                nc.vector.tensor_scalar(
                    vext[:, sk, :, 0:64],
                    ps[:].rearrange("p (h d) -> p h d", h=4),
                    1.0 / WS, None, ALU.mult)

            et_live = {}

            def emit_scores(j, pr, c):
                # one sk-chunk, both heads side by side: hh0 -> [lo:512],
                # hh1 -> [512+lo:1024]; one exp covers both via a strided
                # view; diagonal triangles zeroed by bf16 mtri multiplies
                r = c - 4 * j
                lo = 0 if r < 0 else r * 128
                sp = pss.tile([128, 1024], F32, tag="s", name=f"s{j}{pr}{c}")
                for hh in range(2):
                    hp = slice(64 * hh, 64 * hh + 64)
                    nc.tensor.matmul(
                        sp[:, 512 * hh + lo:512 * hh + 512],
                        kT[hp, pr, c * 128:(c + 1) * 128],
                        qT[hp, pr, j * 512 + lo:(j + 1) * 512],
                        start=True, stop=True)
                et = expp.tile([128, 1024], BF16, tag="et", name=f"e{j}{pr}{c}")
                if lo:
                    ev = et.rearrange("p (t f) -> p t f", t=2)
                    sv = sp.rearrange("p (t f) -> p t f", t=2)
                    nc.scalar.activation(ev[:, :, lo:512], sv[:, :, lo:512],
                                         AF.Exp)
                else:
                    nc.scalar.activation(et[:], sp[:], AF.Exp)
                if r >= 0:
                    # masks on the Pool engine: keeps them out of the DVE
                    # queue, which carries the latency-critical division ops
                    nc.gpsimd.tensor_mul(
                        et[:, lo:lo + 128], et[:, lo:lo + 128], mtri_sb[:])
                    nc.gpsimd.tensor_mul(
                        et[:, 512 + lo:512 + lo + 128],
                        et[:, 512 + lo:512 + lo + 128], mtri_sb[:])
                et_live[(pr, c)] = (et, lo)

            zps_live = {}

            def emit_z(j, pr, c):
                nsk = 4 * (j + 1)
                et, lo = et_live.pop((pr, c))
                if c == 0:
                    zps_live[pr] = [
                        psz.tile([65, 512], F32, tag="z", name=f"z{j}{pr}{hh}")
                        for hh in range(2)]
                zps = zps_live[pr]
                for hh in range(2):
                    nc.tensor.matmul(
                        zps[hh][:, lo:512], vext[:, c, 2 * pr + hh, :],
                        et[:, 512 * hh + lo:512 * hh + 512],
                        start=(c == 0), stop=(c == nsk - 1))

            def emit_division(j, pr):
                js = slice(j * 512, (j + 1) * 512)
                zps = zps_live.pop(pr)
                # denominator rows at partitions 0/32; K=33 ones-matmul
                # broadcasts h0 -> rows 0-63, h1 -> rows 64-127
                rows = rowp.tile([33, 512], F32R, tag="row", name=f"r{j}{pr}")
                nc.vector.tensor_copy(rows[0:1, :], zps[0][64:65, :])
                nc.vector.tensor_copy(rows[32:33, :], zps[1][64:65, :])
                bcp = psw.tile([128, 512], F32, tag="w", name=f"b{j}{pr}")
                nc.tensor.matmul(bcp[:], on2_sb[:], rows[:],
                                 start=True, stop=True)
                rb = rbp.tile([128, 512], F32, tag="rb", name=f"rb{j}{pr}")
                nc.vector.reciprocal(rb[:], bcp[:])
                for hh in range(2):
                    hp = slice(64 * hh, 64 * hh + 64)
                    nc.vector.tensor_mul(zT[hp, pr, js], zps[hh][0:64, :],
                                         rb[hp, :])

            def emit_out(j, c):
                js = slice(j * 512, (j + 1) * 512)
                ops = psw.tile([128, 512], F32, tag="w", name=f"o{j}{c}")
                for pr in range(2):
                    nc.tensor.matmul(
                        ops[:], wo_sb[:, pr, c, :], zT[:, pr, js],
                        start=(pr == 0), stop=(pr == 1))
                ob = outp.tile([128, 512], F32, tag="ob", name=f"ob{j}{c}")
                nc.vector.tensor_copy(ob[:], ops[:])
                nc.sync.dma_start(out=ot[c, :, js], in_=ob[:])

            # ---------- deadline-driven scheduler ----------
            import heapq
            Q = []
            seq = [0]
            BIG = 1 << 30

            def key(j, pr, c):
                return (j * 2 + pr) * 100 + c

            def push(dl, fn):
                seq[0] += 1
                heapq.heappush(Q, (dl, seq[0], fn))

            # deadlines sit one context EARLY (-150/-120/-60) so boundary
            # work (next band's q/k) is emitted before the division chain
            # serializes the pr transition
            for jj in range(NB):
                for pr in range(2):
                    push(key(jj, pr, 0) - 150,
                         (lambda a=jj, b=pr: emit_qk(a, b, True)))
                    push(key(jj, pr, 4 * jj) - 120,
                         (lambda a=jj, b=pr: emit_qk(a, b, False)))
                for sl in range(4):
                    push(key(jj, 0, 4 * jj + sl) - 60,
                         (lambda a=jj, b=sl: emit_v(a, b)))

            credit = [0.0]
            rem_pumps = [sum(2 * (2 * 4 * (j + 1) + 1) for j in range(NB))]

            def pump(pos, force=0):
                while Q and Q[0][0] <= pos:
                    heapq.heappop(Q)[2]()
                rate = min(2.0, len(Q) / max(1, rem_pumps[0]))
                rem_pumps[0] -= 1
                credit[0] += rate
                n = 0
                while Q and (credit[0] >= 1.0 or n < force):
                    heapq.heappop(Q)[2]()
                    if credit[0] >= 1.0:
                        credit[0] -= 1.0
                    n += 1

            def flush_div(pending):
                dj, dpr = pending
                emit_division(dj, dpr)
                if dpr == 1:
                    # BIG deadline: the heap then prefers every projection
                    # unit over out units, so outs drift to the filler-
                    # starved late bands on their own
                    for c in range(NC):
                        push(BIG, (lambda a=dj, b=c: emit_out(a, b)))

            pending_div = None
            for j in range(NB):
                G = 4 * (j + 1)
                for pr in range(2):
                    pump(key(j, pr, 0))
                    emit_scores(j, pr, 0)
                    if pending_div is not None:
                        flush_div(pending_div)
                        pending_div = None
                    for g in range(G):
                        if g + 1 < G:
                            nxt = g + 1
                            diag = (nxt - 4 * j) >= 0
                            pump(key(j, pr, nxt), force=1 if diag else 0)
                            emit_scores(j, pr, nxt)
                        else:
                            pump(key(j, pr, g))
                        emit_z(j, pr, g)
                        pump(key(j, pr, g))
                    pending_div = (j, pr)
            flush_div(pending_div)
            while Q:
                heapq.heappop(Q)[2]()

    nc.compile()
    return nc


def _mtri():
    p = np.arange(128)[:, None]
    f = np.arange(128)[None, :]
    return (f >= p)


def _ones2():
    o = np.zeros((33, 128), np.float32)
    o[0, 0:64] = 1.0
    o[32, 64:128] = 1.0
    return o


_XT_CACHE = {}
_W_CACHE = {}


def _split8(a):
    import ml_dtypes
    E4 = ml_dtypes.float8_e4m3
    hi = np.asarray(a, np.float32).astype(E4)
    lo = (np.asarray(a, np.float32) - hi.astype(np.float32)).astype(E4)
    return hi, lo


def _prep_core(core, x, W_Q, W_K, W_V, W_O, b_Q, b_K):
    import ml_dtypes
    BF = ml_dtypes.bfloat16
    b, g = divmod(core, 4)
    h0 = 4 * g
    key = id(x)
    if (key, b) not in _XT_CACHE:
        if len(_XT_CACHE) > 8:
            _XT_CACHE.clear()
        xT = np.ascontiguousarray(x[b].T)                 # [D, S]
        xp = np.ascontiguousarray(xT.reshape(NC, 128, S).transpose(1, 0, 2))
        _XT_CACHE[(key, b)] = _split8(xp)
    xh, xl = _XT_CACHE[(key, b)]

    wkey = (id(W_Q), g)
    if wkey not in _W_CACHE:
        if len(_W_CACHE) > 8:
            _W_CACHE.clear()

        def pack_qk(W):
            out = np.empty((128, 2, 4, 2, 128), np.float32)
            for pr in range(2):
                Wp = WS * W[h0 + 2 * pr:h0 + 2 * pr + 2].reshape(128, D)
                # [m, D] -> [p, kp, kt, m]
                out[:, pr] = Wp.T.reshape(4, 2, 128, 128).transpose(2, 0, 1, 3)
            return _split8(out)

        Wv4 = WS * W_V[h0:h0 + 4].reshape(256, D)         # [n, D]
        wv = np.ascontiguousarray(
            Wv4.T.reshape(4, 2, 128, 256).transpose(2, 0, 1, 3))

        wo = np.empty((128, 2, NC, 128), np.float32)
        for pr in range(2):
            Wp = W_O[h0 + 2 * pr:h0 + 2 * pr + 2]         # [2, D, 64]
            arr = Wp.transpose(0, 2, 1).reshape(128, D)   # [128(k), D]
            wo[:, pr] = arr.reshape(128, NC, 128)
        wo = np.ascontiguousarray(wo)

        _W_CACHE[wkey] = (pack_qk(W_Q), pack_qk(W_K), _split8(wv), wo)
    (wqh, wql), (wkh, wkl), (wvh, wvl), wo = _W_CACHE[wkey]

    bqv = np.stack([b_Q[h0 + 2 * pr:h0 + 2 * pr + 2].reshape(128) / ATTN_SCALE
                    for pr in range(2)], axis=1).astype(np.float32)
    bkv = np.stack([b_K[h0 + 2 * pr:h0 + 2 * pr + 2].reshape(128)
                    for pr in range(2)], axis=1).astype(np.float32)

    return dict(
        xh=xh, xl=xl, wqh=wqh, wql=wql, wkh=wkh, wkl=wkl,
        wvh=wvh, wvl=wvl, wo=wo,
        bq=bqv, bk=bkv,
        ones2=_ones2(),
        mtri=_mtri().astype(BF),
        onesv=np.ones((128, NSK, 4, 1), np.float32).astype(BF),
    )


def kernel(x, W_Q, W_K, W_V, W_O, b_Q, b_K, b_V, b_O):
    global _COMPILED
    from concourse.bass_utils import run_bass_kernel_spmd

    x = np.asarray(x, np.float32)
    W_Q = np.asarray(W_Q, np.float32)
    W_K = np.asarray(W_K, np.float32)
    W_V = np.asarray(W_V, np.float32)
    W_O = np.asarray(W_O, np.float32)
    b_Q = np.asarray(b_Q, np.float32)
    b_K = np.asarray(b_K, np.float32)
    b_V = np.asarray(b_V, np.float32)
    b_O = np.asarray(b_O, np.float32)

    if _COMPILED is None:
        _COMPILED = _build_program()
    nc = _COMPILED

    in_maps = [_prep_core(c, x, W_Q, W_K, W_V, W_O, b_Q, b_K)
               for c in range(N_CORES)]
    res = run_bass_kernel_spmd(nc, in_maps, core_ids=list(range(N_CORES)))

    # host gather: sum head-group partials, add folded output bias, transpose
    bias_total = b_O + np.einsum('idh,ih->d', W_O, b_V)
    out = np.empty((B, S, D), np.float32)
    for b in range(B):
        acc = res.results[4 * b]["ot"].astype(np.float64)
        for g in range(1, 4):
            acc += res.results[4 * b + g]["ot"]
        out[b] = acc.reshape(D, S).T + bias_total
    return out


# revision 18
# speedup vs baseline: 1.0683x; 1.0351x over previous
"""Self-contained TRN2 Bass kernel for the causal multi-head attention problem.

Problem (hardcoded): B=2, S=2048, D=1024, H=16, DH=64, fp32, causal.
Sharding: 8 cores = 2 batches x 4 head-groups of 4 heads each.

Per-core strategy ("T layout": feature dim on partitions, sequence on free):
  x ships as an fp8 hi+lo pair (host split); Q/K/V projections run as
  error-compensated fp8 DoubleRow matmuls (W_hi*x_hi + W_hi*x_lo + W_lo*x_hi
  into one fp32 PSUM group, K=256 per matmul) -- 2x fewer PE cycles than
  fp32r at ~bf16 accuracy. Attention runs bf16 (qT/kT/et/vext) at exact
  causal widths; output projection stays fp32r. Scores for one sk-chunk put
  both heads of a pair side by side in a [128,1024] PSUM tile so one
  activation does the exp for both; the sk>sq triangles of diagonal chunks
  are zeroed by 0/1 bf16 multiplies (head 0 on DVE for latency, head 1 on
  the otherwise-idle Pool engine). Softmax denominator rides the z matmul
  as a ones-column; division via K=33 ones-matmul broadcast + DVE
  reciprocal + multiply.

Scheduling: scores run one sk-chunk ahead of z (skew) so the ACT-engine exp
latency is hidden, and a deadline-driven filler queue interleaves projection
and output-projection PSUM groups between attention chunks (with priority
at diagonal chunks, whose exp->mask->z chains are longest) so the PE never
idles. Host folds: 1/sqrt(DH) and the fp8 weight scale into the PSUM
evacuation scale, b_Q/b_K into the evacuation bias, b_V and b_O into a
single host-side output bias (rows of attn sum to 1); final partial sums
over the 4 head-group cores on the host.
"""

import numpy as np

B, S, D = 2, 2048, 1024
H, DH = 16, 64
ATTN_SCALE = 8.0  # sqrt(64)
WS = 64.0         # fp8 weight pre-scale (folded out at PSUM evacuation)
N_CORES = 8
NC = D // 128          # 8 D-chunks
NB = S // 512          # 4 sq bands
NSK = S // 128         # 16 sk chunks

_COMPILED = None


def _build_program():
    import concourse.mybir as mybir
    import concourse.tile as tile
    from concourse import bacc

    F32 = mybir.dt.float32
    F32R = mybir.dt.float32r
    BF16 = mybir.dt.bfloat16
    FP8 = mybir.dt.float8e4
    DR = mybir.MatmulPerfMode.DoubleRow
    AF = mybir.ActivationFunctionType
    ALU = mybir.AluOpType

    nc = bacc.Bacc("TRN2", target_bir_lowering=False, debug=False,
                   num_devices=N_CORES)

    xh = nc.dram_tensor("xh", [128, NC, S], FP8, kind="ExternalInput")
    xl = nc.dram_tensor("xl", [128, NC, S], FP8, kind="ExternalInput")
    wqh = nc.dram_tensor("wqh", [128, 2, 4, 2, 128], FP8, kind="ExternalInput")
    wql = nc.dram_tensor("wql", [128, 2, 4, 2, 128], FP8, kind="ExternalInput")
    wkh = nc.dram_tensor("wkh", [128, 2, 4, 2, 128], FP8, kind="ExternalInput")
    wkl = nc.dram_tensor("wkl", [128, 2, 4, 2, 128], FP8, kind="ExternalInput")
    wvh = nc.dram_tensor("wvh", [128, 4, 2, 256], FP8, kind="ExternalInput")
    wvl = nc.dram_tensor("wvl", [128, 4, 2, 256], FP8, kind="ExternalInput")
    wo = nc.dram_tensor("wo", [128, 2, NC, 128], F32R, kind="ExternalInput")
    bq = nc.dram_tensor("bq", [128, 2], F32, kind="ExternalInput")
    bk = nc.dram_tensor("bk", [128, 2], F32, kind="ExternalInput")
    ones2 = nc.dram_tensor("ones2", [33, 128], F32R, kind="ExternalInput")
    onesv = nc.dram_tensor("onesv", [128, NSK, 4, 1], BF16, kind="ExternalInput")
    mtri = nc.dram_tensor("mtri", [128, 128], BF16, kind="ExternalInput")
    ot = nc.dram_tensor("ot", [NC, 128, S], F32, kind="ExternalOutput")

    with tile.TileContext(nc) as tc:
        with (
            tc.tile_pool(name="const", bufs=1) as cst,
            tc.tile_pool(name="xbp", bufs=8) as xbp,
            tc.tile_pool(name="qkz", bufs=1) as qkz,
            tc.tile_pool(name="expp", bufs=6) as expp,
            tc.tile_pool(name="rowp", bufs=3) as rowp,
            tc.tile_pool(name="rbp", bufs=3) as rbp,
            tc.tile_pool(name="outp", bufs=4) as outp,
            tc.tile_pool(name="pss", bufs=2, space="PSUM") as pss,
            tc.tile_pool(name="psw", bufs=2, space="PSUM") as psw,
            tc.tile_pool(name="psz", bufs=2, space="PSUM") as psz,
        ):
            wqh_sb = cst.tile([128, 2, 4, 2, 128], FP8)
            wql_sb = cst.tile([128, 2, 4, 2, 128], FP8)
            wkh_sb = cst.tile([128, 2, 4, 2, 128], FP8)
            wkl_sb = cst.tile([128, 2, 4, 2, 128], FP8)
            wvh_sb = cst.tile([128, 4, 2, 256], FP8)
            wvl_sb = cst.tile([128, 4, 2, 256], FP8)
            wo_sb = cst.tile([128, 2, NC, 128], F32R)
            bq_sb = cst.tile([128, 2], F32)
            bk_sb = cst.tile([128, 2], F32)
            on2_sb = cst.tile([33, 128], F32R)
            mtri_sb = cst.tile([128, 128], BF16)
            xhb = [xbp.tile([128, NC, 512], FP8, name=f"xhb{j}", tag="xb")
                   for j in range(NB)]
            xlb = [xbp.tile([128, NC, 512], FP8, name=f"xlb{j}", tag="xb")
                   for j in range(NB)]
            qT = qkz.tile([128, 2, S], BF16)    # [2 heads of pair x dh, pr, s]
            kT = qkz.tile([128, 2, S], BF16)
            vext = qkz.tile([128, NSK, 4, 65], BF16)  # [sk, chunk, head, dh|1]
            zT = qkz.tile([128, 2, S], F32R)

            # warm the PE (p-state) and the ACT exp table while the input
            # DMAs are in flight; results are discarded
            wu_w = cst.tile([128, 128], F32)
            wu_r = cst.tile([128, 512], F32)
            wu_o = cst.tile([128, 512], F32)
            nc.vector.memset(wu_w[:], 0.0)
            nc.vector.memset(wu_r[:], 0.0)
            wup = psw.tile([128, 512], F32, tag="w", name="wup")
            for _i in range(8):
                nc.tensor.matmul(wup[:], wu_w[:], wu_r[:],
                                 start=(_i == 0), stop=(_i == 7))
            nc.scalar.activation(wu_o[:], wu_r[:], AF.Exp)

            # DMA order: first-proj-unit critical path first
            nc.sync.dma_start(out=wqh_sb[:], in_=wqh[:])
            nc.sync.dma_start(out=xhb[0][:], in_=xh[:, :, 0:512])
            nc.sync.dma_start(out=wql_sb[:], in_=wql[:])
            nc.sync.dma_start(out=xlb[0][:], in_=xl[:, :, 0:512])
            nc.sync.dma_start(out=wkh_sb[:], in_=wkh[:])
            nc.sync.dma_start(out=wkl_sb[:], in_=wkl[:])
            nc.sync.dma_start(out=bq_sb[:], in_=bq[:])
            nc.sync.dma_start(out=bk_sb[:], in_=bk[:])
            nc.sync.dma_start(out=wvh_sb[:], in_=wvh[:])
            nc.sync.dma_start(out=wvl_sb[:], in_=wvl[:])
            nc.sync.dma_start(out=mtri_sb[:], in_=mtri[:])
            nc.sync.dma_start(out=on2_sb[:], in_=ones2[:])
            nc.sync.dma_start(out=vext[:, :, :, 64:65], in_=onesv[:])
            for j in range(1, NB):
                nc.sync.dma_start(out=xhb[j][:], in_=xh[:, :, j * 512:(j + 1) * 512])
                nc.sync.dma_start(out=xlb[j][:], in_=xl[:, :, j * 512:(j + 1) * 512])
            nc.sync.dma_start(out=wo_sb[:], in_=wo[:])

            # ---------- emit-unit definitions ----------

            def emit_qk(jj, pr, is_q):
                js = slice(jj * 512, (jj + 1) * 512)
                wh, wl = (wqh_sb, wql_sb) if is_q else (wkh_sb, wkl_sb)
                ps = psw.tile([128, 512], F32, tag="w",
                              name=f"qk{jj}{pr}{int(is_q)}")
                i = 0
                for w_sb, x_b in ((wh, xhb[jj]), (wh, xlb[jj]), (wl, xhb[jj])):
                    for kp in range(4):
                        nc.tensor.matmul(
                            ps[:], w_sb[:, pr, kp],
                            x_b[:, 2 * kp:2 * kp + 2, :],
                            start=(i == 0), stop=(i == 11), perf_mode=DR,
                        )
                        i += 1
                if is_q:
                    nc.vector.tensor_scalar(
                        qT[:, pr, js], ps[:], 1.0 / (WS * ATTN_SCALE),
                        bq_sb[:, pr:pr + 1], ALU.mult, ALU.add)
                else:
                    nc.vector.tensor_scalar(
                        kT[:, pr, js], ps[:], 1.0 / WS,
                        bk_sb[:, pr:pr + 1], ALU.mult, ALU.add)

            def emit_v(jj, sl):
                sk = 4 * jj + sl
                ss = slice(sl * 128, (sl + 1) * 128)
                ps = psw.tile([128, 256], F32, tag="w", name=f"v{jj}{sl}")
                i = 0
                for x_b, w_sb in ((xhb[jj], wvh_sb), (xlb[jj], wvh_sb),
                                  (xhb[jj], wvl_sb)):
                    for kp in range(4):
                        nc.tensor.matmul(
                            ps[:], x_b[:, 2 * kp:2 * kp + 2, ss],
                            w_sb[:, kp], start=(i == 0), stop=(i == 11),
                            perf_mode=DR,
                        )
                        i += 1
                nc.vector.tensor_scalar(
                    vext[:, sk, :, 0:64],
                    ps[:].rearrange("p (h d) -> p h d", h=4),
                    1.0 / WS, None, ALU.mult)

            et_live = {}

            def emit_scores(j, pr, c):
                # one sk-chunk, both heads side by side: hh0 -> [lo:512],
                # hh1 -> [512+lo:1024]; one exp covers both via a strided
                # view; diagonal triangles zeroed by bf16 mtri multiplies
                r = c - 4 * j
                lo = 0 if r < 0 else r * 128
                sp = pss.tile([128, 1024], F32, tag="s", name=f"s{j}{pr}{c}")
                for hh in range(2):
                    hp = slice(64 * hh, 64 * hh + 64)
                    nc.tensor.matmul(
                        sp[:, 512 * hh + lo:512 * hh + 512],
                        kT[hp, pr, c * 128:(c + 1) * 128],
                        qT[hp, pr, j * 512 + lo:(j + 1) * 512],
                        start=True, stop=True)
                et = expp.tile([128, 1024], BF16, tag="et", name=f"e{j}{pr}{c}")
                if lo:
                    ev = et.rearrange("p (t f) -> p t f", t=2)
                    sv = sp.rearrange("p (t f) -> p t f", t=2)
                    nc.scalar.activation(ev[:, :, lo:512], sv[:, :, lo:512],
                                         AF.Exp)
                else:
                    nc.scalar.activation(et[:], sp[:], AF.Exp)
                if r >= 0:
                    # hh0's mask on DVE (shortest latency to its z matmul),
                    # hh1's on the Pool engine (hidden behind hh0's z)
                    nc.vector.tensor_mul(
                        et[:, lo:lo + 128], et[:, lo:lo + 128], mtri_sb[:])
                    nc.gpsimd.tensor_mul(
                        et[:, 512 + lo:512 + lo + 128],
                        et[:, 512 + lo:512 + lo + 128], mtri_sb[:])
                et_live[(pr, c)] = (et, lo)

            zps_live = {}

            def emit_z(j, pr, c):
                nsk = 4 * (j + 1)
                et, lo = et_live.pop((pr, c))
                if c == 0:
                    zps_live[pr] = [
                        psz.tile([65, 512], F32, tag="z", name=f"z{j}{pr}{hh}")
                        for hh in range(2)]
                zps = zps_live[pr]
                for hh in range(2):
                    nc.tensor.matmul(
                        zps[hh][:, lo:512], vext[:, c, 2 * pr + hh, :],
                        et[:, 512 * hh + lo:512 * hh + 512],
                        start=(c == 0), stop=(c == nsk - 1))

            rows_live = {}

            def emit_div_rows(j, pr):
                # DVE-side denominator gather: emitted right at context end
                # so the DVE starts while the PE moves on
                zps = zps_live.pop(pr)
                rows = rowp.tile([33, 512], F32R, tag="row", name=f"r{j}{pr}")
                nc.vector.tensor_copy(rows[0:1, :], zps[0][64:65, :])
                nc.vector.tensor_copy(rows[32:33, :], zps[1][64:65, :])
                rows_live[(j, pr)] = (rows, zps)

            def emit_div_pe(j, pr):
                # the PE-visible part (bcp matmul) is deferred a few units
                # into the next context: the in-order PE would otherwise
                # stall on the rows copies at every band/pr boundary
                js = slice(j * 512, (j + 1) * 512)
                rows, zps = rows_live.pop((j, pr))
                bcp = psw.tile([128, 512], F32, tag="w", name=f"b{j}{pr}")
                nc.tensor.matmul(bcp[:], on2_sb[:], rows[:],
                                 start=True, stop=True)
                rb = rbp.tile([128, 512], F32, tag="rb", name=f"rb{j}{pr}")
                nc.vector.reciprocal(rb[:], bcp[:])
                for hh in range(2):
                    hp = slice(64 * hh, 64 * hh + 64)
                    nc.vector.tensor_mul(zT[hp, pr, js], zps[hh][0:64, :],
                                         rb[hp, :])

            def emit_out(j, c):
                js = slice(j * 512, (j + 1) * 512)
                ops = psw.tile([128, 512], F32, tag="w", name=f"o{j}{c}")
                for pr in range(2):
                    nc.tensor.matmul(
                        ops[:], wo_sb[:, pr, c, :], zT[:, pr, js],
                        start=(pr == 0), stop=(pr == 1))
                ob = outp.tile([128, 512], F32, tag="ob", name=f"ob{j}{c}")
                nc.vector.tensor_copy(ob[:], ops[:])
                nc.sync.dma_start(out=ot[c, :, js], in_=ob[:])

            # ---------- deadline-driven scheduler ----------
            import heapq
            Q = []
            seq = [0]
            BIG = 1 << 30

            def key(j, pr, c):
                return (j * 2 + pr) * 100 + c

            def push(dl, fn):
                seq[0] += 1
                heapq.heappush(Q, (dl, seq[0], fn))

            # deadlines sit one context EARLY (-150/-120/-60) so boundary
            # work (next band's q/k) is emitted before the division chain
            # serializes the pr transition
            for jj in range(NB):
                for pr in range(2):
                    push(key(jj, pr, 0) - 150,
                         (lambda a=jj, b=pr: emit_qk(a, b, True)))
                    push(key(jj, pr, 4 * jj) - 120,
                         (lambda a=jj, b=pr: emit_qk(a, b, False)))
                for sl in range(4):
                    push(key(jj, 0, 4 * jj + sl) - 60,
                         (lambda a=jj, b=sl: emit_v(a, b)))

            credit = [0.0]
            rem_pumps = [sum(2 * (2 * 4 * (j + 1) + 1) for j in range(NB))]

            def pump(pos, force=0):
                while Q and Q[0][0] <= pos:
                    heapq.heappop(Q)[2]()
                rate = min(2.0, len(Q) / max(1, rem_pumps[0]))
                rem_pumps[0] -= 1
                credit[0] += rate
                n = 0
                while Q and (credit[0] >= 1.0 or n < force):
                    heapq.heappop(Q)[2]()
                    if credit[0] >= 1.0:
                        credit[0] -= 1.0
                    n += 1

            def flush_div(pending):
                dj, dpr = pending
                emit_div_pe(dj, dpr)
                if dpr == 1:
                    # BIG deadline: the heap then prefers every projection
                    # unit over out units, so outs drift to the filler-
                    # starved late bands on their own
                    for c in range(NC):
                        push(BIG, (lambda a=dj, b=c: emit_out(a, b)))

            pending_div = None
            div_delay = [0]
            for j in range(NB):
                G = 4 * (j + 1)
                for pr in range(2):
                    pump(key(j, pr, 0))
                    emit_scores(j, pr, 0)
                    for g in range(G):
                        if g + 1 < G:
                            nxt = g + 1
                            diag = (nxt - 4 * j) >= 0
                            pump(key(j, pr, nxt), force=1 if diag else 0)
                            emit_scores(j, pr, nxt)
                        else:
                            pump(key(j, pr, g))
                        emit_z(j, pr, g)
                        pump(key(j, pr, g))
                        if pending_div is not None:
                            div_delay[0] -= 1
                            if div_delay[0] <= 0:
                                flush_div(pending_div)
                                pending_div = None
                    if pending_div is not None:
                        flush_div(pending_div)
                        pending_div = None
                    emit_div_rows(j, pr)
                    pending_div = (j, pr)
                    div_delay[0] = 2
            flush_div(pending_div)
            while Q:
                heapq.heappop(Q)[2]()

    nc.compile()
    return nc


def _mtri():
    p = np.arange(128)[:, None]
    f = np.arange(128)[None, :]
    return (f >= p)


def _ones2():
    o = np.zeros((33, 128), np.float32)
    o[0, 0:64] = 1.0
    o[32, 64:128] = 1.0
    return o


_XT_CACHE = {}
_W_CACHE = {}


def _split8(a):
    import ml_dtypes
    E4 = ml_dtypes.float8_e4m3
    hi = np.asarray(a, np.float32).astype(E4)
    lo = (np.asarray(a, np.float32) - hi.astype(np.float32)).astype(E4)
    return hi, lo


def _prep_core(core, x, W_Q, W_K, W_V, W_O, b_Q, b_K):
    import ml_dtypes
    BF = ml_dtypes.bfloat16
    b, g = divmod(core, 4)
    h0 = 4 * g
    key = id(x)
    if (key, b) not in _XT_CACHE:
        if len(_XT_CACHE) > 8:
            _XT_CACHE.clear()
        xT = np.ascontiguousarray(x[b].T)                 # [D, S]
        xp = np.ascontiguousarray(xT.reshape(NC, 128, S).transpose(1, 0, 2))
        _XT_CACHE[(key, b)] = _split8(xp)
    xh, xl = _XT_CACHE[(key, b)]

    wkey = (id(W_Q), g)
    if wkey not in _W_CACHE:
        if len(_W_CACHE) > 8:
            _W_CACHE.clear()

        def pack_qk(W):
            out = np.empty((128, 2, 4, 2, 128), np.float32)
            for pr in range(2):
                Wp = WS * W[h0 + 2 * pr:h0 + 2 * pr + 2].reshape(128, D)
                # [m, D] -> [p, kp, kt, m]
                out[:, pr] = Wp.T.reshape(4, 2, 128, 128).transpose(2, 0, 1, 3)
            return _split8(out)

        Wv4 = WS * W_V[h0:h0 + 4].reshape(256, D)         # [n, D]
        wv = np.ascontiguousarray(
            Wv4.T.reshape(4, 2, 128, 256).transpose(2, 0, 1, 3))

        wo = np.empty((128, 2, NC, 128), np.float32)
        for pr in range(2):
            Wp = W_O[h0 + 2 * pr:h0 + 2 * pr + 2]         # [2, D, 64]
            arr = Wp.transpose(0, 2, 1).reshape(128, D)   # [128(k), D]
            wo[:, pr] = arr.reshape(128, NC, 128)
        wo = np.ascontiguousarray(wo)

        _W_CACHE[wkey] = (pack_qk(W_Q), pack_qk(W_K), _split8(wv), wo)
    (wqh, wql), (wkh, wkl), (wvh, wvl), wo = _W_CACHE[wkey]

    bqv = np.stack([b_Q[h0 + 2 * pr:h0 + 2 * pr + 2].reshape(128) / ATTN_SCALE
                    for pr in range(2)], axis=1).astype(np.float32)
    bkv = np.stack([b_K[h0 + 2 * pr:h0 + 2 * pr + 2].reshape(128)
                    for pr in range(2)], axis=1).astype(np.float32)

    return dict(
        xh=xh, xl=xl, wqh=wqh, wql=wql, wkh=wkh, wkl=wkl,
        wvh=wvh, wvl=wvl, wo=wo,
        bq=bqv, bk=bkv,
        ones2=_ones2(),
        mtri=_mtri().astype(BF),
        onesv=np.ones((128, NSK, 4, 1), np.float32).astype(BF),
    )


def kernel(x, W_Q, W_K, W_V, W_O, b_Q, b_K, b_V, b_O):
    global _COMPILED
    from concourse.bass_utils import run_bass_kernel_spmd

    x = np.asarray(x, np.float32)
    W_Q = np.asarray(W_Q, np.float32)
    W_K = np.asarray(W_K, np.float32)
    W_V = np.asarray(W_V, np.float32)
    W_O = np.asarray(W_O, np.float32)
    b_Q = np.asarray(b_Q, np.float32)
    b_K = np.asarray(b_K, np.float32)
    b_V = np.asarray(b_V, np.float32)
    b_O = np.asarray(b_O, np.float32)

    if _COMPILED is None:
        _COMPILED = _build_program()
    nc = _COMPILED

    in_maps = [_prep_core(c, x, W_Q, W_K, W_V, W_O, b_Q, b_K)
               for c in range(N_CORES)]
    res = run_bass_kernel_spmd(nc, in_maps, core_ids=list(range(N_CORES)))

    # host gather: sum head-group partials, add folded output bias, transpose
    bias_total = b_O + np.einsum('idh,ih->d', W_O, b_V)
    out = np.empty((B, S, D), np.float32)
    for b in range(B):
        acc = res.results[4 * b]["ot"].astype(np.float64)
        for g in range(1, 4):
            acc += res.results[4 * b + g]["ot"]
        out[b] = acc.reshape(D, S).T + bias_total
    return out
